# revision 6
# baseline (speedup 1.0000x reference)
"""TRN2 Bass kernel for nn_CompiledBlock_45148696216108 (moe_routing).

Reference computation:
    xp  = x[permute]
    xn  = LayerNorm(xp; gamma, beta, eps=1e-5)
    yp  = xn @ W.T + b
    out = (xp + yp)[argsort(permute)]

The block (LayerNorm + Linear + residual) is purely row-wise, so the
gather by `permute` and the scatter by its inverse cancel exactly:
    out = x + LN(x) @ W.T + b
No token movement (and no cross-core all-to-all) is needed. Tokens are
sharded contiguously across the 8 cores; the tiny weights are folded on
the host and replicated:
    A[h, o] = gamma[h] * W[o, h]          (pre-transposed, gamma folded)
    c[o]    = sum_h beta[h] * W[o, h] + b[o]
    out     = x + ((x - mu) * rsig) @ A + c

Per-core device pipeline (tokens_per_core = 8192, HIDDEN = 2048):
  - DMA x tile (128 tokens, 2048) in natural layout
  - DVE bn_stats/bn_aggr -> mean/var; ACT sqrt + DVE reciprocal -> rsig
  - ACT: xn = (x - mu) * rsig  (per-partition scale/bias)
  - PE transposes xn into (h, t) layout (fp32, via identity)
  - PE matmuls in float32r (full-rate fp32 with ~1.5e-4 rel precision):
    psum[t, o] = sum_k xnT[k].T @ A[k]  (+ bias row via K=1 matmul)
  - DVE: out = x + psum; DMA out
"""

import numpy as np
from contextlib import ExitStack

from concourse import bacc, tile, mybir
from concourse.bass_utils import run_bass_kernel_spmd
from concourse.masks import make_identity

N_TOK = 65536
HIDDEN = 2048
N_CORES = 8
P = 128
EPS = 1e-5
F32 = mybir.dt.float32
F32R = mybir.dt.float32r
AF = mybir.ActivationFunctionType
ALU = mybir.AluOpType


def build(tokens_per_core: int = N_TOK // N_CORES, num_devices: int = N_CORES):
    T = tokens_per_core
    NT = T // P            # token tiles
    KC = HIDDEN // P       # contraction chunks of 128
    NB = 512               # matmul free dim (one PSUM bank of fp32)
    OC = HIDDEN // NB      # output column chunks

    nc = bacc.Bacc(
        "TRN2", target_bir_lowering=False, debug=False, num_devices=num_devices
    )
    x_d = nc.dram_tensor("x", [T, HIDDEN], F32, kind="ExternalInput").ap()
    a_d = nc.dram_tensor("A", [KC, P, HIDDEN], F32R, kind="ExternalInput").ap()
    c_d = nc.dram_tensor("c", [1, HIDDEN], F32R, kind="ExternalInput").ap()
    out_d = nc.dram_tensor("out", [T, HIDDEN], F32, kind="ExternalOutput").ap()

    with tile.TileContext(nc) as tc, ExitStack() as ctx:
        const = ctx.enter_context(tc.tile_pool(name="const", bufs=1))
        apool = ctx.enter_context(tc.tile_pool(name="apool", bufs=1))
        xpool = ctx.enter_context(tc.tile_pool(name="xpool", bufs=2))
        xnpool = ctx.enter_context(tc.tile_pool(name="xnpool", bufs=2))
        xtpool = ctx.enter_context(tc.tile_pool(name="xtpool", bufs=2))
        outpool = ctx.enter_context(tc.tile_pool(name="outpool", bufs=2))
        stpool = ctx.enter_context(tc.tile_pool(name="stats", bufs=3))
        psy_pool = ctx.enter_context(tc.tile_pool(name="psy", bufs=1, space="PSUM"))
        pst_pool = ctx.enter_context(tc.tile_pool(name="pst", bufs=2, space="PSUM"))

        # Resident weights: A as 16 chunks of (128 h, 2048 o)
        a_sb = apool.tile([P, KC, HIDDEN], F32R)
        for k in range(KC):
            nc.sync.dma_start(a_sb[:, k, :], a_d[k])
        c_sb = const.tile([1, HIDDEN], F32R)
        nc.sync.dma_start(c_sb[:], c_d[:])
        ones_f32 = const.tile([1, P], F32)
        nc.gpsimd.memset(ones_f32[:], 1.0)
        ones = const.tile([1, P], F32R)
        nc.vector.tensor_copy(ones[:], ones_f32[:])
        ident = const.tile([P, P], F32)
        make_identity(nc, ident[:])
        eps_sb = const.tile([P, 1], F32)
        nc.gpsimd.memset(eps_sb[:], EPS)

        for t in range(NT):
            xt = xpool.tile([P, HIDDEN], F32)
            nc.sync.dma_start(xt[:], x_d[t * P : (t + 1) * P, :])

            # LayerNorm statistics
            stats = stpool.tile([P, 4, 6], F32)
            xr = xt[:].rearrange("p (a b) -> p a b", b=512)
            for a in range(4):
                nc.vector.bn_stats(stats[:, a, :], xr[:, a, :])
            mv = stpool.tile([P, 2], F32)
            nc.vector.bn_aggr(mv[:], stats[:])
            sig = stpool.tile([P, 1], F32)
            nc.scalar.activation(sig[:], mv[:, 1:2], AF.Sqrt, bias=eps_sb[:])
            rsig = stpool.tile([P, 1], F32)
            nc.vector.reciprocal(rsig[:], sig[:])
            nmr = stpool.tile([P, 1], F32)  # -mu * rsig
            nc.vector.scalar_tensor_tensor(
                nmr[:], mv[:, 0:1], -1.0, rsig[:], ALU.mult, ALU.mult
            )

            # xn = (x - mu) * rsig
            xn = xnpool.tile([P, HIDDEN], F32)
            nc.scalar.activation(
                xn[:], xt[:], AF.Identity, bias=nmr[:], scale=rsig[:]
            )

            # Transpose xn -> xnT (h on partitions), 4 chunks per PSUM bank
            xnt = xtpool.tile([P, KC, P], F32R)
            for g in range(KC // 4):
                pst = pst_pool.tile([P, 4 * P], F32)
                for j in range(4):
                    k = 4 * g + j
                    nc.tensor.transpose(
                        pst[:, j * P : (j + 1) * P],
                        xn[:, k * P : (k + 1) * P],
                        ident[:],
                    )
                nc.vector.tensor_copy(xnt[:, 4 * g : 4 * g + 4, :], pst[:])

            # psum[t, o] = bias + sum_k xnT[k].T @ A[k]
            psy = psy_pool.tile([P, HIDDEN], F32)
            for j in range(OC):
                nc.tensor.matmul(
                    psy[:, j * NB : (j + 1) * NB],
                    ones[:],
                    c_sb[:, j * NB : (j + 1) * NB],
                    start=True,
                    stop=False,
                )
            for k in range(KC):
                lhsT = xnt[:, k, :]
                for j in range(OC):
                    nc.tensor.matmul(
                        psy[:, j * NB : (j + 1) * NB],
                        lhsT,
                        a_sb[:, k, j * NB : (j + 1) * NB],
                        start=False,
                        stop=(k == KC - 1),
                    )

            # out = x + psum ; store
            ot = outpool.tile([P, HIDDEN], F32)
            nc.vector.tensor_add(ot[:], psy[:], xt[:])
            nc.sync.dma_start(out_d[t * P : (t + 1) * P, :], ot[:])

    nc.compile()
    return nc


_built = None


def _get_built():
    global _built
    if _built is None:
        _built = build()
    return _built


def _prep_inputs(x, permute, gamma, beta, W, b):
    x = np.asarray(x, dtype=np.float32)
    gamma = np.asarray(gamma, dtype=np.float32)
    beta = np.asarray(beta, dtype=np.float32)
    W = np.asarray(W, dtype=np.float32)
    b = np.asarray(b, dtype=np.float32)
    A = np.ascontiguousarray(W.T) * gamma[:, None]  # (H, O), gamma folded
    A = np.ascontiguousarray(A.reshape(HIDDEN // P, P, HIDDEN))
    c = (W @ beta + b).reshape(1, HIDDEN)
    T = N_TOK // N_CORES
    in_maps = []
    for i in range(N_CORES):
        in_maps.append({"x": x[i * T : (i + 1) * T], "A": A, "c": c})
    return in_maps


def kernel(x, permute, gamma, beta, W, b):
    nc = _get_built()
    in_maps = _prep_inputs(x, permute, gamma, beta, W, b)
    res = run_bass_kernel_spmd(nc, in_maps, list(range(N_CORES))).results
    return np.concatenate([r["out"] for r in res], axis=0)


if __name__ == "__main__":
    rng = np.random.default_rng(0)
    x = rng.standard_normal((N_TOK, HIDDEN), dtype=np.float32)
    permute = rng.permutation(N_TOK).astype(np.int64)
    gamma = np.ones(HIDDEN, np.float32)
    beta = np.zeros(HIDDEN, np.float32)
    W = (rng.standard_normal((HIDDEN, HIDDEN), dtype=np.float32) / np.sqrt(HIDDEN))
    b = rng.standard_normal(HIDDEN, dtype=np.float32) * 0.01
    out = kernel(x=x, permute=permute, gamma=gamma, beta=beta, W=W, b=b)
    print(out.shape, out.dtype)


# revision 10
# speedup vs baseline: 1.2638x; 1.2638x over previous
"""TRN2 Bass kernel for nn_CompiledBlock_45148696216108 (moe_routing).

Reference computation:
    xp  = x[permute]
    xn  = LayerNorm(xp; gamma, beta, eps=1e-5)
    yp  = xn @ W.T + b
    out = (xp + yp)[argsort(permute)]

The block (LayerNorm + Linear + residual) is purely row-wise, so the
gather by `permute` and the scatter by its inverse cancel exactly:
    out = x + LN(x) @ W.T + b
No token movement (and no cross-core all-to-all) is needed. Tokens are
sharded contiguously across the 8 cores; the tiny weights are folded on
the host and replicated:
    A[h, o] = gamma[h] * W[o, h]          (pre-transposed, gamma folded)
    c[o]    = sum_h beta[h] * W[o, h] + b[o]
    out     = x + ((x - mu) * rsig) @ A + c

Per-core device pipeline (tokens_per_core = 8192, HIDDEN = 2048):
  - DMA x tile (128 tokens, 2048) in natural layout
  - DVE bn_stats/bn_aggr -> mean/var; ACT sqrt + DVE reciprocal -> rsig
  - ACT: xn = (x - mu) * rsig  (per-partition scale/bias)
  - PE transposes xn into (h, t) layout (fp32, via identity)
  - PE matmuls in float32r (full-rate fp32, ~1e-4 rel err):
    psum_j[t, o512] = bias + sum_k xnT[k].T @ A[k, j]
  - DVE per-bank: out_j = x_j + psum_j; DMA out
Stats/normalize for tile i+1 are emitted before tile i's matmul phase so
ACT/DVE prep overlaps the PE matmul stream (keeps PE dense / HAM warm).
"""

import numpy as np
from contextlib import ExitStack

from concourse import bacc, tile, mybir
from concourse.bass_utils import run_bass_kernel_spmd
from concourse.masks import make_identity

N_TOK = 65536
HIDDEN = 2048
N_CORES = 8
P = 128
EPS = 1e-5
F32 = mybir.dt.float32
F32R = mybir.dt.float32r
AF = mybir.ActivationFunctionType
ALU = mybir.AluOpType

NB = 512               # matmul free dim (one PSUM bank of fp32)
KC = HIDDEN // P       # 16 contraction chunks
OC = HIDDEN // NB      # 4 output column chunks


def build(tokens_per_core: int = N_TOK // N_CORES, num_devices: int = N_CORES):
    T = tokens_per_core
    NT = T // P            # token tiles

    nc = bacc.Bacc(
        "TRN2", target_bir_lowering=False, debug=False, num_devices=num_devices
    )
    x_d = nc.dram_tensor("x", [T, HIDDEN], F32, kind="ExternalInput").ap()
    a_d = nc.dram_tensor("A", [KC, P, HIDDEN], F32R, kind="ExternalInput").ap()
    c_d = nc.dram_tensor("c", [1, HIDDEN], F32R, kind="ExternalInput").ap()
    out_d = nc.dram_tensor("out", [T, HIDDEN], F32, kind="ExternalOutput").ap()

    with tile.TileContext(nc) as tc, ExitStack() as ctx:
        const = ctx.enter_context(tc.tile_pool(name="const", bufs=1))
        apool = ctx.enter_context(tc.tile_pool(name="apool", bufs=1))
        xpool = ctx.enter_context(tc.tile_pool(name="xpool", bufs=3))
        xnpool = ctx.enter_context(tc.tile_pool(name="xnpool", bufs=2))
        xtpool = ctx.enter_context(tc.tile_pool(name="xtpool", bufs=1))
        outpool = ctx.enter_context(tc.tile_pool(name="outpool", bufs=2))
        stpool = ctx.enter_context(tc.tile_pool(name="stats", bufs=3))
        psy_pool = ctx.enter_context(tc.tile_pool(name="psy", bufs=1, space="PSUM"))
        pst_pool = ctx.enter_context(tc.tile_pool(name="pst", bufs=2, space="PSUM"))

        # Prefetch the first x tiles BEFORE the 16 MB weight DMA so the
        # LN/normalize prologue runs during the weight load.
        xts = {}
        for i in range(min(2, NT)):
            xts[i] = xpool.tile([P, HIDDEN], F32, tag="xt", name=f"xt_pre{i}")
            nc.sync.dma_start(xts[i][:], x_d[i * P : (i + 1) * P, :])

        # Resident weights: 16 separately-tracked chunks of (128 h, 2048 o)
        # so matmuls on chunk k only wait for chunk k's DMA.
        a_sb = []
        for k in range(KC):
            ak = apool.tile([P, HIDDEN], F32R, tag=f"a{k}")
            nc.sync.dma_start(ak[:], a_d[k])
            a_sb.append(ak)
        c_sb = const.tile([1, HIDDEN], F32R)
        nc.sync.dma_start(c_sb[:], c_d[:])
        ones_f32 = const.tile([1, P], F32)
        nc.gpsimd.memset(ones_f32[:], 1.0)
        ones = const.tile([1, P], F32R)
        nc.vector.tensor_copy(ones[:], ones_f32[:])
        ident = const.tile([P, P], F32)
        make_identity(nc, ident[:])
        eps_sb = const.tile([P, 1], F32)
        nc.gpsimd.memset(eps_sb[:], EPS)

        def stats_and_norm(i, xt):
            """LN stats + normalize for tile i -> xn tile (ACT/DVE work)."""
            stats = stpool.tile([P, 4, 6], F32, tag="stats")
            xr = xt[:].rearrange("p (a b) -> p a b", b=512)
            for a in range(4):
                nc.vector.bn_stats(stats[:, a, :], xr[:, a, :])
            mv = stpool.tile([P, 2], F32, tag="mv")
            nc.vector.bn_aggr(mv[:], stats[:])
            sig = stpool.tile([P, 1], F32, tag="sig")
            nc.scalar.activation(sig[:], mv[:, 1:2], AF.Sqrt, bias=eps_sb[:])
            rsig = stpool.tile([P, 1], F32, tag="rsig")
            nc.vector.reciprocal(rsig[:], sig[:])
            nmr = stpool.tile([P, 1], F32, tag="nmr")
            nc.vector.scalar_tensor_tensor(
                nmr[:], mv[:, 0:1], -1.0, rsig[:], ALU.mult, ALU.mult
            )
            xn = xnpool.tile([P, HIDDEN], F32, tag="xn")
            nc.scalar.activation(xn[:], xt[:], AF.Identity, bias=nmr[:], scale=rsig[:])
            return xn

        xns = {}
        xns[0] = stats_and_norm(0, xts[0])

        for t in range(NT):
            xt = xts.pop(t)
            xn = xns.pop(t)

            # Transpose xn -> xnT (h on partitions), 4 chunks per PSUM bank
            xnt = xtpool.tile([P, KC, P], F32R, tag="xnt")
            for g in range(KC // 4):
                pst = pst_pool.tile([P, 4 * P], F32, tag="pst")
                for j in range(4):
                    k = 4 * g + j
                    nc.tensor.transpose(
                        pst[:, j * P : (j + 1) * P],
                        xn[:, k * P : (k + 1) * P],
                        ident[:],
                    )
                nc.vector.tensor_copy(xnt[:, 4 * g : 4 * g + 4, :], pst[:])

            # Prefetch + prep next tile while this tile's matmuls run
            if t + 2 < NT:
                xts[t + 2] = xpool.tile([P, HIDDEN], F32, tag="xt", name=f"xt_{t + 2}")
                nc.sync.dma_start(
                    xts[t + 2][:], x_d[(t + 2) * P : (t + 3) * P, :]
                )
            if t + 1 < NT:
                xns[t + 1] = stats_and_norm(t + 1, xts[t + 1])  # noqa

            # Matmuls, one PSUM bank (512 outputs) at a time; combine per bank
            ot = outpool.tile([P, HIDDEN], F32, tag="ot")
            for j in range(OC):
                psy = psy_pool.tile([P, NB], F32, tag=f"psy{j}")
                sl = slice(j * NB, (j + 1) * NB)
                nc.tensor.matmul(
                    psy[:], ones[:], c_sb[:, sl], start=True, stop=False
                )
                for k in range(KC):
                    nc.tensor.matmul(
                        psy[:],
                        xnt[:, k, :],
                        a_sb[k][:, sl],
                        start=False,
                        stop=(k == KC - 1),
                    )
                nc.vector.tensor_add(ot[:, sl], psy[:], xt[:, sl])

            nc.sync.dma_start(out_d[t * P : (t + 1) * P, :], ot[:])

    nc.compile()
    return nc


_built = None


def _get_built():
    global _built
    if _built is None:
        _built = build()
    return _built


def _prep_inputs(x, permute, gamma, beta, W, b):
    x = np.asarray(x, dtype=np.float32)
    gamma = np.asarray(gamma, dtype=np.float32)
    beta = np.asarray(beta, dtype=np.float32)
    W = np.asarray(W, dtype=np.float32)
    b = np.asarray(b, dtype=np.float32)
    A = np.ascontiguousarray(W.T) * gamma[:, None]  # (H, O), gamma folded
    A = np.ascontiguousarray(A.reshape(HIDDEN // P, P, HIDDEN))
    c = (W @ beta + b).reshape(1, HIDDEN)
    T = N_TOK // N_CORES
    in_maps = []
    for i in range(N_CORES):
        in_maps.append({"x": x[i * T : (i + 1) * T], "A": A, "c": c})
    return in_maps


def kernel(x, permute, gamma, beta, W, b):
    nc = _get_built()
    in_maps = _prep_inputs(x, permute, gamma, beta, W, b)
    res = run_bass_kernel_spmd(nc, in_maps, list(range(N_CORES))).results
    return np.concatenate([r["out"] for r in res], axis=0)


if __name__ == "__main__":
    rng = np.random.default_rng(0)
    x = rng.standard_normal((N_TOK, HIDDEN), dtype=np.float32)
    permute = rng.permutation(N_TOK).astype(np.int64)
    gamma = np.ones(HIDDEN, np.float32)
    beta = np.zeros(HIDDEN, np.float32)
    W = (rng.standard_normal((HIDDEN, HIDDEN), dtype=np.float32) / np.sqrt(HIDDEN))
    b = rng.standard_normal(HIDDEN, dtype=np.float32) * 0.01
    out = kernel(x=x, permute=permute, gamma=gamma, beta=beta, W=W, b=b)
    print(out.shape, out.dtype)


# revision 11
# speedup vs baseline: 1.3096x; 1.0363x over previous
"""TRN2 Bass kernel for nn_CompiledBlock_45148696216108 (moe_routing).

Reference computation:
    xp  = x[permute]
    xn  = LayerNorm(xp; gamma, beta, eps=1e-5)
    yp  = xn @ W.T + b
    out = (xp + yp)[argsort(permute)]

The block (LayerNorm + Linear + residual) is purely row-wise, so the
gather by `permute` and the scatter by its inverse cancel exactly:
    out = x + LN(x) @ W.T + b
No token movement (and no cross-core all-to-all) is needed. Tokens are
sharded contiguously across the 8 cores; the tiny weights are folded on
the host and replicated:
    A[h, o] = gamma[h] * W[o, h]          (pre-transposed, gamma folded)
    c[o]    = sum_h beta[h] * W[o, h] + b[o]
    out     = x + ((x - mu) * rsig) @ A + c

Per-core device pipeline (tokens_per_core = 8192, HIDDEN = 2048):
  - DMA x tile (128 tokens, 2048) in natural layout
  - DVE bn_stats/bn_aggr -> mean/var; ACT sqrt + DVE reciprocal -> rsig
  - ACT: xn = (x - mu) * rsig  (per-partition scale/bias)
  - PE transposes xn into (h, t) layout (fp32, via identity)
  - PE matmuls in float32r (full-rate fp32, ~1e-4 rel err):
    psum_j[t, o512] = bias + sum_k xnT[k].T @ A[k, j]
  - DVE per-bank: out_j = x_j + psum_j; DMA out
Stats/normalize for tile i+1 are emitted before tile i's matmul phase so
ACT/DVE prep overlaps the PE matmul stream (keeps PE dense / HAM warm).
"""

import numpy as np
from contextlib import ExitStack

from concourse import bacc, tile, mybir
from concourse.bass_utils import run_bass_kernel_spmd
from concourse.masks import make_identity

N_TOK = 65536
HIDDEN = 2048
N_CORES = 8
P = 128
EPS = 1e-5
F32 = mybir.dt.float32
F32R = mybir.dt.float32r
AF = mybir.ActivationFunctionType
ALU = mybir.AluOpType

NB = 512               # matmul free dim (one PSUM bank of fp32)
KC = HIDDEN // P       # 16 contraction chunks
OC = HIDDEN // NB      # 4 output column chunks


def build(tokens_per_core: int = N_TOK // N_CORES, num_devices: int = N_CORES):
    T = tokens_per_core
    NT = T // P            # token tiles

    nc = bacc.Bacc(
        "TRN2", target_bir_lowering=False, debug=False, num_devices=num_devices
    )
    x_d = nc.dram_tensor("x", [T, HIDDEN], F32, kind="ExternalInput").ap()
    a_d = nc.dram_tensor("A", [KC, P, HIDDEN], F32R, kind="ExternalInput").ap()
    c_d = nc.dram_tensor("c", [P, HIDDEN], F32, kind="ExternalInput").ap()
    out_d = nc.dram_tensor("out", [T, HIDDEN], F32, kind="ExternalOutput").ap()

    with tile.TileContext(nc) as tc, ExitStack() as ctx:
        const = ctx.enter_context(tc.tile_pool(name="const", bufs=1))
        apool = ctx.enter_context(tc.tile_pool(name="apool", bufs=1))
        xpool = ctx.enter_context(tc.tile_pool(name="xpool", bufs=2))
        xnpool = ctx.enter_context(tc.tile_pool(name="xnpool", bufs=2))
        xtpool = ctx.enter_context(tc.tile_pool(name="xtpool", bufs=1))
        outpool = ctx.enter_context(tc.tile_pool(name="outpool", bufs=2))
        stpool = ctx.enter_context(tc.tile_pool(name="stats", bufs=3))
        psy_pool = ctx.enter_context(tc.tile_pool(name="psy", bufs=1, space="PSUM"))
        pst_pool = ctx.enter_context(tc.tile_pool(name="pst", bufs=2, space="PSUM"))

        # Prefetch the first x tiles BEFORE the 16 MB weight DMA so the
        # LN/normalize prologue runs during the weight load.
        c_sb = const.tile([P, HIDDEN], F32)
        nc.sync.dma_start(c_sb[:], c_d[:])
        xts = {}
        for i in range(min(2, NT)):
            xts[i] = xpool.tile([P, HIDDEN], F32, tag="xt", name=f"xt_pre{i}")
            nc.sync.dma_start(xts[i][:], x_d[i * P : (i + 1) * P, :])

        # Resident weights: 16 separately-tracked chunks of (128 h, 2048 o)
        # so matmuls on chunk k only wait for chunk k's DMA.
        a_sb = []
        for k in range(KC):
            ak = apool.tile([P, HIDDEN], F32R, tag=f"a{k}")
            nc.sync.dma_start(ak[:], a_d[k])
            a_sb.append(ak)
        ident = const.tile([P, P], F32)
        make_identity(nc, ident[:])
        eps_sb = const.tile([P, 1], F32)
        nc.gpsimd.memset(eps_sb[:], EPS)

        def stats_and_norm(i, xt):
            """LN stats + normalize for tile i -> xn tile (ACT/DVE work)."""
            stats = stpool.tile([P, 4, 6], F32, tag="stats")
            xr = xt[:].rearrange("p (a b) -> p a b", b=512)
            for a in range(4):
                nc.vector.bn_stats(stats[:, a, :], xr[:, a, :])
            mv = stpool.tile([P, 2], F32, tag="mv")
            nc.vector.bn_aggr(mv[:], stats[:])
            sig = stpool.tile([P, 1], F32, tag="sig")
            nc.scalar.activation(sig[:], mv[:, 1:2], AF.Sqrt, bias=eps_sb[:])
            rsig = stpool.tile([P, 1], F32, tag="rsig")
            nc.vector.reciprocal(rsig[:], sig[:])
            nmr = stpool.tile([P, 1], F32, tag="nmr")
            nc.vector.scalar_tensor_tensor(
                nmr[:], mv[:, 0:1], -1.0, rsig[:], ALU.mult, ALU.mult
            )
            xn = xnpool.tile([P, HIDDEN], F32, tag="xn")
            nc.scalar.activation(xn[:], xt[:], AF.Identity, bias=nmr[:], scale=rsig[:])
            # Pre-bias the residual in place (WAR on the norm read above):
            # combine later does out = psy + (x + c) in one DVE pass.
            nc.gpsimd.tensor_add(xt[:], xt[:], c_sb[:])
            return xn

        xns = {}
        xns[0] = stats_and_norm(0, xts[0])

        for t in range(NT):
            xt = xts.pop(t)
            xn = xns.pop(t)

            # Transpose xn -> xnT (h on partitions), 4 chunks per PSUM bank
            xnt = xtpool.tile([P, KC, P], F32R, tag="xnt")
            for g in range(KC // 4):
                pst = pst_pool.tile([P, 4 * P], F32, tag="pst")
                for j in range(4):
                    k = 4 * g + j
                    nc.tensor.transpose(
                        pst[:, j * P : (j + 1) * P],
                        xn[:, k * P : (k + 1) * P],
                        ident[:],
                    )
                nc.vector.tensor_copy(xnt[:, 4 * g : 4 * g + 4, :], pst[:])

            # Prefetch + prep next tile while this tile's matmuls run
            if t + 2 < NT:
                xts[t + 2] = xpool.tile([P, HIDDEN], F32, tag="xt", name=f"xt_{t + 2}")
                nc.sync.dma_start(
                    xts[t + 2][:], x_d[(t + 2) * P : (t + 3) * P, :]
                )
            if t + 1 < NT:
                xns[t + 1] = stats_and_norm(t + 1, xts[t + 1])  # noqa

            # Matmuls, one PSUM bank (512 outputs) at a time; combine per bank
            ot = outpool.tile([P, HIDDEN], F32, tag="ot")
            for j in range(OC):
                psy = psy_pool.tile([P, NB], F32, tag=f"psy{j}")
                sl = slice(j * NB, (j + 1) * NB)
                for k in range(KC):
                    nc.tensor.matmul(
                        psy[:],
                        xnt[:, k, :],
                        a_sb[k][:, sl],
                        start=(k == 0),
                        stop=(k == KC - 1),
                    )
                nc.vector.tensor_add(ot[:, sl], psy[:], xt[:, sl])

            nc.sync.dma_start(out_d[t * P : (t + 1) * P, :], ot[:])

    nc.compile()
    return nc


_built = None


def _get_built():
    global _built
    if _built is None:
        _built = build()
    return _built


def _prep_inputs(x, permute, gamma, beta, W, b):
    x = np.asarray(x, dtype=np.float32)
    gamma = np.asarray(gamma, dtype=np.float32)
    beta = np.asarray(beta, dtype=np.float32)
    W = np.asarray(W, dtype=np.float32)
    b = np.asarray(b, dtype=np.float32)
    A = np.ascontiguousarray(W.T) * gamma[:, None]  # (H, O), gamma folded
    A = np.ascontiguousarray(A.reshape(HIDDEN // P, P, HIDDEN))
    c = np.ascontiguousarray(
        np.broadcast_to((W @ beta + b).reshape(1, HIDDEN), (P, HIDDEN))
    ).astype(np.float32)
    T = N_TOK // N_CORES
    in_maps = []
    for i in range(N_CORES):
        in_maps.append({"x": x[i * T : (i + 1) * T], "A": A, "c": c})
    return in_maps


def kernel(x, permute, gamma, beta, W, b):
    nc = _get_built()
    in_maps = _prep_inputs(x, permute, gamma, beta, W, b)
    res = run_bass_kernel_spmd(nc, in_maps, list(range(N_CORES))).results
    return np.concatenate([r["out"] for r in res], axis=0)


if __name__ == "__main__":
    rng = np.random.default_rng(0)
    x = rng.standard_normal((N_TOK, HIDDEN), dtype=np.float32)
    permute = rng.permutation(N_TOK).astype(np.int64)
    gamma = np.ones(HIDDEN, np.float32)
    beta = np.zeros(HIDDEN, np.float32)
    W = (rng.standard_normal((HIDDEN, HIDDEN), dtype=np.float32) / np.sqrt(HIDDEN))
    b = rng.standard_normal(HIDDEN, dtype=np.float32) * 0.01
    out = kernel(x=x, permute=permute, gamma=gamma, beta=beta, W=W, b=b)
    print(out.shape, out.dtype)


# revision 12
# speedup vs baseline: 1.3320x; 1.0171x over previous
"""TRN2 Bass kernel for nn_CompiledBlock_45148696216108 (moe_routing).

Reference computation:
    xp  = x[permute]
    xn  = LayerNorm(xp; gamma, beta, eps=1e-5)
    yp  = xn @ W.T + b
    out = (xp + yp)[argsort(permute)]

The block (LayerNorm + Linear + residual) is purely row-wise, so the
gather by `permute` and the scatter by its inverse cancel exactly:
    out = x + LN(x) @ W.T + b
No token movement (and no cross-core all-to-all) is needed. Tokens are
sharded contiguously across the 8 cores; the tiny weights are folded on
the host and replicated:
    A[h, o] = gamma[h] * W[o, h]          (pre-transposed, gamma folded)
    c[o]    = sum_h beta[h] * W[o, h] + b[o]
    out     = x + ((x - mu) * rsig) @ A + c

Per-core device pipeline (tokens_per_core = 8192, HIDDEN = 2048):
  - DMA x tile (128 tokens, 2048) in natural layout
  - DVE bn_stats/bn_aggr -> mean/var; ACT sqrt + DVE reciprocal -> rsig
  - ACT: xn = (x - mu) * rsig  (per-partition scale/bias)
  - PE transposes xn into (h, t) layout (fp32, via identity)
  - PE matmuls in float32r (full-rate fp32, ~1e-4 rel err):
    psum_j[t, o512] = bias + sum_k xnT[k].T @ A[k, j]
  - DVE per-bank: out_j = x_j + psum_j; DMA out
Stats/normalize for tile i+1 are emitted before tile i's matmul phase so
ACT/DVE prep overlaps the PE matmul stream (keeps PE dense / HAM warm).
"""

import numpy as np
from contextlib import ExitStack

from concourse import bacc, tile, mybir
from concourse.bass_utils import run_bass_kernel_spmd
from concourse.masks import make_identity

N_TOK = 65536
HIDDEN = 2048
N_CORES = 8
P = 128
EPS = 1e-5
F32 = mybir.dt.float32
F32R = mybir.dt.float32r
AF = mybir.ActivationFunctionType
ALU = mybir.AluOpType

NB = 512               # matmul free dim (one PSUM bank of fp32)
KC = HIDDEN // P       # 16 contraction chunks
OC = HIDDEN // NB      # 4 output column chunks


def build(tokens_per_core: int = N_TOK // N_CORES, num_devices: int = N_CORES):
    T = tokens_per_core
    NT = T // P            # token tiles

    nc = bacc.Bacc(
        "TRN2", target_bir_lowering=False, debug=False, num_devices=num_devices
    )
    x_d = nc.dram_tensor("x", [T, HIDDEN], F32, kind="ExternalInput").ap()
    a_d = nc.dram_tensor("A", [KC, P, HIDDEN], F32R, kind="ExternalInput").ap()
    c_d = nc.dram_tensor("c", [P, HIDDEN], F32, kind="ExternalInput").ap()
    out_d = nc.dram_tensor("out", [T, HIDDEN], F32, kind="ExternalOutput").ap()

    with tile.TileContext(nc) as tc, ExitStack() as ctx:
        const = ctx.enter_context(tc.tile_pool(name="const", bufs=1))
        apool = ctx.enter_context(tc.tile_pool(name="apool", bufs=1))
        xpool = ctx.enter_context(tc.tile_pool(name="xpool", bufs=2))
        xnpool = ctx.enter_context(tc.tile_pool(name="xnpool", bufs=2))
        xtpool = ctx.enter_context(tc.tile_pool(name="xtpool", bufs=1))
        outpool = ctx.enter_context(tc.tile_pool(name="outpool", bufs=2))
        stpool = ctx.enter_context(tc.tile_pool(name="stats", bufs=3))
        psy_pool = ctx.enter_context(tc.tile_pool(name="psy", bufs=1, space="PSUM"))
        pst_pool = ctx.enter_context(tc.tile_pool(name="pst", bufs=3, space="PSUM"))

        # Prefetch the first x tiles BEFORE the 16 MB weight DMA so the
        # LN/normalize prologue runs during the weight load.
        c_sb = const.tile([P, HIDDEN], F32)
        nc.sync.dma_start(c_sb[:], c_d[:])
        xts = {}
        for i in range(min(2, NT)):
            xts[i] = xpool.tile([P, HIDDEN], F32, tag="xt", name=f"xt_pre{i}")
            nc.sync.dma_start(xts[i][:], x_d[i * P : (i + 1) * P, :])

        # Resident weights: 16 separately-tracked chunks of (128 h, 2048 o)
        # so matmuls on chunk k only wait for chunk k's DMA.
        a_sb = []
        for k in range(KC):
            ak = apool.tile([P, HIDDEN], F32R, tag=f"a{k}")
            nc.sync.dma_start(ak[:], a_d[k])
            a_sb.append(ak)
        ident_f32 = const.tile([P, P], F32)
        make_identity(nc, ident_f32[:])
        ident = const.tile([P, P], F32R)
        nc.vector.tensor_copy(ident[:], ident_f32[:])
        eps_sb = const.tile([P, 1], F32)
        nc.gpsimd.memset(eps_sb[:], EPS)

        def stats_and_norm(i, xt):
            """LN stats + normalize for tile i -> xn tile (ACT/DVE work)."""
            stats = stpool.tile([P, 4, 6], F32, tag="stats")
            xr = xt[:].rearrange("p (a b) -> p a b", b=512)
            for a in range(4):
                nc.vector.bn_stats(stats[:, a, :], xr[:, a, :])
            mv = stpool.tile([P, 2], F32, tag="mv")
            nc.vector.bn_aggr(mv[:], stats[:])
            sig = stpool.tile([P, 1], F32, tag="sig")
            nc.scalar.activation(sig[:], mv[:, 1:2], AF.Sqrt, bias=eps_sb[:])
            rsig = stpool.tile([P, 1], F32, tag="rsig")
            nc.vector.reciprocal(rsig[:], sig[:])
            nmr = stpool.tile([P, 1], F32, tag="nmr")
            nc.vector.scalar_tensor_tensor(
                nmr[:], mv[:, 0:1], -1.0, rsig[:], ALU.mult, ALU.mult
            )
            xn = xnpool.tile([P, HIDDEN], F32R, tag="xn")
            nc.scalar.activation(xn[:], xt[:], AF.Identity, bias=nmr[:], scale=rsig[:])
            # Pre-bias the residual in place (WAR on the norm read above):
            # combine later does out = psy + (x + c) in one DVE pass.
            nc.gpsimd.tensor_add(xt[:], xt[:], c_sb[:])
            return xn

        xns = {}
        xns[0] = stats_and_norm(0, xts[0])

        for t in range(NT):
            xt = xts.pop(t)
            xn = xns.pop(t)

            # Transpose xn -> xnT (h on partitions), 4 chunks per PSUM bank
            xnt = xtpool.tile([P, KC, P], F32R, tag="xnt")
            for g in range(KC // 4):
                pst = pst_pool.tile([P, 4 * P], F32R, tag="pst")
                for j in range(4):
                    k = 4 * g + j
                    nc.tensor.transpose(
                        pst[:, j * P : (j + 1) * P],
                        xn[:, k * P : (k + 1) * P],
                        ident[:],
                    )
                nc.vector.tensor_copy(xnt[:, 4 * g : 4 * g + 4, :], pst[:])

            # Prefetch + prep next tile while this tile's matmuls run
            if t + 2 < NT:
                xts[t + 2] = xpool.tile([P, HIDDEN], F32, tag="xt", name=f"xt_{t + 2}")
                nc.sync.dma_start(
                    xts[t + 2][:], x_d[(t + 2) * P : (t + 3) * P, :]
                )
            if t + 1 < NT:
                xns[t + 1] = stats_and_norm(t + 1, xts[t + 1])  # noqa

            # Matmuls, one PSUM bank (512 outputs) at a time; combine per bank
            ot = outpool.tile([P, HIDDEN], F32, tag="ot")
            for j in range(OC):
                psy = psy_pool.tile([P, NB], F32, tag=f"psy{j}")
                sl = slice(j * NB, (j + 1) * NB)
                for k in range(KC):
                    nc.tensor.matmul(
                        psy[:],
                        xnt[:, k, :],
                        a_sb[k][:, sl],
                        start=(k == 0),
                        stop=(k == KC - 1),
                    )
                nc.vector.tensor_add(ot[:, sl], psy[:], xt[:, sl])

            nc.sync.dma_start(out_d[t * P : (t + 1) * P, :], ot[:])

    nc.compile()
    return nc


_built = None


def _get_built():
    global _built
    if _built is None:
        _built = build()
    return _built


def _prep_inputs(x, permute, gamma, beta, W, b):
    x = np.asarray(x, dtype=np.float32)
    gamma = np.asarray(gamma, dtype=np.float32)
    beta = np.asarray(beta, dtype=np.float32)
    W = np.asarray(W, dtype=np.float32)
    b = np.asarray(b, dtype=np.float32)
    A = np.ascontiguousarray(W.T) * gamma[:, None]  # (H, O), gamma folded
    A = np.ascontiguousarray(A.reshape(HIDDEN // P, P, HIDDEN))
    c = np.ascontiguousarray(
        np.broadcast_to((W @ beta + b).reshape(1, HIDDEN), (P, HIDDEN))
    ).astype(np.float32)
    T = N_TOK // N_CORES
    in_maps = []
    for i in range(N_CORES):
        in_maps.append({"x": x[i * T : (i + 1) * T], "A": A, "c": c})
    return in_maps


def kernel(x, permute, gamma, beta, W, b):
    nc = _get_built()
    in_maps = _prep_inputs(x, permute, gamma, beta, W, b)
    res = run_bass_kernel_spmd(nc, in_maps, list(range(N_CORES))).results
    return np.concatenate([r["out"] for r in res], axis=0)


if __name__ == "__main__":
    rng = np.random.default_rng(0)
    x = rng.standard_normal((N_TOK, HIDDEN), dtype=np.float32)
    permute = rng.permutation(N_TOK).astype(np.int64)
    gamma = np.ones(HIDDEN, np.float32)
    beta = np.zeros(HIDDEN, np.float32)
    W = (rng.standard_normal((HIDDEN, HIDDEN), dtype=np.float32) / np.sqrt(HIDDEN))
    b = rng.standard_normal(HIDDEN, dtype=np.float32) * 0.01
    out = kernel(x=x, permute=permute, gamma=gamma, beta=beta, W=W, b=b)
    print(out.shape, out.dtype)


# revision 13
# speedup vs baseline: 1.3383x; 1.0048x over previous
"""TRN2 Bass kernel for nn_CompiledBlock_45148696216108 (moe_routing).

Reference computation:
    xp  = x[permute]
    xn  = LayerNorm(xp; gamma, beta, eps=1e-5)
    yp  = xn @ W.T + b
    out = (xp + yp)[argsort(permute)]

The block (LayerNorm + Linear + residual) is purely row-wise, so the
gather by `permute` and the scatter by its inverse cancel exactly:
    out = x + LN(x) @ W.T + b
No token movement (and no cross-core all-to-all) is needed. Tokens are
sharded contiguously across the 8 cores; the tiny weights are folded on
the host and replicated:
    A[h, o] = gamma[h] * W[o, h]          (pre-transposed, gamma folded)
    c[o]    = sum_h beta[h] * W[o, h] + b[o]
    out     = x + ((x - mu) * rsig) @ A + c

Per-core device pipeline (tokens_per_core = 8192, HIDDEN = 2048):
  - DMA x tile (128 tokens, 2048) in natural layout
  - DVE bn_stats/bn_aggr -> mean/var; ACT sqrt + DVE reciprocal -> rsig
  - ACT: xn = (x - mu) * rsig  (per-partition scale/bias)
  - PE transposes xn into (h, t) layout (fp32, via identity)
  - PE matmuls in float32r (full-rate fp32, ~1e-4 rel err):
    psum_j[t, o512] = bias + sum_k xnT[k].T @ A[k, j]
  - DVE per-bank: out_j = x_j + psum_j; DMA out
Stats/normalize for tile i+1 are emitted before tile i's matmul phase so
ACT/DVE prep overlaps the PE matmul stream (keeps PE dense / HAM warm).
"""

import numpy as np
from contextlib import ExitStack

from concourse import bacc, tile, mybir
from concourse.bass_utils import run_bass_kernel_spmd
from concourse.masks import make_identity

N_TOK = 65536
HIDDEN = 2048
N_CORES = 8
P = 128
EPS = 1e-5
F32 = mybir.dt.float32
F32R = mybir.dt.float32r
AF = mybir.ActivationFunctionType
ALU = mybir.AluOpType

NB = 512               # matmul free dim (one PSUM bank of fp32)
KC = HIDDEN // P       # 16 contraction chunks
OC = HIDDEN // NB      # 4 output column chunks


def build(tokens_per_core: int = N_TOK // N_CORES, num_devices: int = N_CORES):
    T = tokens_per_core
    NT = T // P            # token tiles

    nc = bacc.Bacc(
        "TRN2", target_bir_lowering=False, debug=False, num_devices=num_devices
    )
    x_d = nc.dram_tensor("x", [T, HIDDEN], F32, kind="ExternalInput").ap()
    a_d = nc.dram_tensor("A", [KC, P, HIDDEN], F32R, kind="ExternalInput").ap()
    c_d = nc.dram_tensor("c", [P, HIDDEN], F32, kind="ExternalInput").ap()
    out_d = nc.dram_tensor("out", [T, HIDDEN], F32, kind="ExternalOutput").ap()

    with tile.TileContext(nc) as tc, ExitStack() as ctx:
        const = ctx.enter_context(tc.tile_pool(name="const", bufs=1))
        apool = ctx.enter_context(tc.tile_pool(name="apool", bufs=1))
        xpool = ctx.enter_context(tc.tile_pool(name="xpool", bufs=2))
        xnpool = ctx.enter_context(tc.tile_pool(name="xnpool", bufs=2))
        xtpool = ctx.enter_context(tc.tile_pool(name="xtpool", bufs=1))
        outpool = ctx.enter_context(tc.tile_pool(name="outpool", bufs=2))
        stpool = ctx.enter_context(tc.tile_pool(name="stats", bufs=3))
        psy_pool = ctx.enter_context(tc.tile_pool(name="psy", bufs=1, space="PSUM"))
        pst_pool = ctx.enter_context(tc.tile_pool(name="pst", bufs=3, space="PSUM"))

        # Prefetch the first x tiles BEFORE the 16 MB weight DMA so the
        # LN/normalize prologue runs during the weight load.
        xts = {}
        xts[0] = xpool.tile([P, HIDDEN], F32, tag="xt", name="xt_pre0")
        nc.sync.dma_start(xts[0][:], x_d[0:P, :])
        c_sb = const.tile([P, HIDDEN], F32)
        nc.sync.dma_start(c_sb[:], c_d[:])
        if NT > 1:
            xts[1] = xpool.tile([P, HIDDEN], F32, tag="xt", name="xt_pre1")
            nc.sync.dma_start(xts[1][:], x_d[P : 2 * P, :])

        # Resident weights: 16 separately-tracked chunks of (128 h, 2048 o)
        # so matmuls on chunk k only wait for chunk k's DMA.
        a_sb = []
        for k in range(KC):
            ak = apool.tile([P, HIDDEN], F32R, tag=f"a{k}")
            nc.sync.dma_start(ak[:], a_d[k])
            a_sb.append(ak)
        ident_f32 = const.tile([P, P], F32)
        make_identity(nc, ident_f32[:])
        ident = const.tile([P, P], F32R)
        nc.vector.tensor_copy(ident[:], ident_f32[:])
        eps_sb = const.tile([P, 1], F32)
        nc.gpsimd.memset(eps_sb[:], EPS)

        def stats_and_norm(i, xt):
            """LN stats + normalize for tile i -> xn tile (ACT/DVE work)."""
            stats = stpool.tile([P, 4, 6], F32, tag="stats")
            xr = xt[:].rearrange("p (a b) -> p a b", b=512)
            for a in range(4):
                nc.vector.bn_stats(stats[:, a, :], xr[:, a, :])
            mv = stpool.tile([P, 2], F32, tag="mv")
            nc.vector.bn_aggr(mv[:], stats[:])
            sig = stpool.tile([P, 1], F32, tag="sig")
            nc.scalar.activation(sig[:], mv[:, 1:2], AF.Sqrt, bias=eps_sb[:])
            rsig = stpool.tile([P, 1], F32, tag="rsig")
            nc.vector.reciprocal(rsig[:], sig[:])
            nmr = stpool.tile([P, 1], F32, tag="nmr")
            nc.vector.scalar_tensor_tensor(
                nmr[:], mv[:, 0:1], -1.0, rsig[:], ALU.mult, ALU.mult
            )
            xn = xnpool.tile([P, HIDDEN], F32R, tag="xn")
            nc.scalar.activation(xn[:], xt[:], AF.Identity, bias=nmr[:], scale=rsig[:])
            # Pre-bias the residual in place (WAR on the norm read above):
            # combine later does out = psy + (x + c) in one DVE pass.
            nc.gpsimd.tensor_add(xt[:], xt[:], c_sb[:])
            return xn

        xns = {}
        xns[0] = stats_and_norm(0, xts[0])

        for t in range(NT):
            xt = xts.pop(t)
            xn = xns.pop(t)

            # Transpose xn -> xnT (h on partitions), 4 chunks per PSUM bank.
            # One xnT tile per group of 4 chunks so the first matmuls only
            # wait on the first group's PSUM->SBUF cast, not all four.
            xnt_g = []
            for g in range(KC // 4):
                pst = pst_pool.tile([P, 4 * P], F32R, tag="pst")
                for j in range(4):
                    k = 4 * g + j
                    nc.tensor.transpose(
                        pst[:, j * P : (j + 1) * P],
                        xn[:, k * P : (k + 1) * P],
                        ident[:],
                    )
                xg = xtpool.tile([P, 4, P], F32R, tag=f"xnt{g}", name=f"xnt_{t}_{g}")
                nc.vector.tensor_copy(xg[:], pst[:])
                xnt_g.append(xg)

            # Prefetch + prep next tile while this tile's matmuls run
            if t + 2 < NT:
                xts[t + 2] = xpool.tile([P, HIDDEN], F32, tag="xt", name=f"xt_{t + 2}")
                nc.sync.dma_start(
                    xts[t + 2][:], x_d[(t + 2) * P : (t + 3) * P, :]
                )
            if t + 1 < NT:
                xns[t + 1] = stats_and_norm(t + 1, xts[t + 1])  # noqa

            # Matmuls, one PSUM bank (512 outputs) at a time; combine per bank
            ot = outpool.tile([P, HIDDEN], F32, tag="ot")
            for j in range(OC):
                psy = psy_pool.tile([P, NB], F32, tag=f"psy{j}")
                sl = slice(j * NB, (j + 1) * NB)
                for k in range(KC):
                    nc.tensor.matmul(
                        psy[:],
                        xnt_g[k // 4][:, k % 4, :],
                        a_sb[k][:, sl],
                        start=(k == 0),
                        stop=(k == KC - 1),
                    )
                nc.vector.tensor_add(ot[:, sl], psy[:], xt[:, sl])

            nc.sync.dma_start(out_d[t * P : (t + 1) * P, :], ot[:])

    nc.compile()
    return nc


_built = None


def _get_built():
    global _built
    if _built is None:
        _built = build()
    return _built


def _prep_inputs(x, permute, gamma, beta, W, b):
    x = np.asarray(x, dtype=np.float32)
    gamma = np.asarray(gamma, dtype=np.float32)
    beta = np.asarray(beta, dtype=np.float32)
    W = np.asarray(W, dtype=np.float32)
    b = np.asarray(b, dtype=np.float32)
    A = np.ascontiguousarray(W.T) * gamma[:, None]  # (H, O), gamma folded
    A = np.ascontiguousarray(A.reshape(HIDDEN // P, P, HIDDEN))
    c = np.ascontiguousarray(
        np.broadcast_to((W @ beta + b).reshape(1, HIDDEN), (P, HIDDEN))
    ).astype(np.float32)
    T = N_TOK // N_CORES
    in_maps = []
    for i in range(N_CORES):
        in_maps.append({"x": x[i * T : (i + 1) * T], "A": A, "c": c})
    return in_maps


def kernel(x, permute, gamma, beta, W, b):
    nc = _get_built()
    in_maps = _prep_inputs(x, permute, gamma, beta, W, b)
    res = run_bass_kernel_spmd(nc, in_maps, list(range(N_CORES))).results
    return np.concatenate([r["out"] for r in res], axis=0)


if __name__ == "__main__":
    rng = np.random.default_rng(0)
    x = rng.standard_normal((N_TOK, HIDDEN), dtype=np.float32)
    permute = rng.permutation(N_TOK).astype(np.int64)
    gamma = np.ones(HIDDEN, np.float32)
    beta = np.zeros(HIDDEN, np.float32)
    W = (rng.standard_normal((HIDDEN, HIDDEN), dtype=np.float32) / np.sqrt(HIDDEN))
    b = rng.standard_normal(HIDDEN, dtype=np.float32) * 0.01
    out = kernel(x=x, permute=permute, gamma=gamma, beta=beta, W=W, b=b)
    print(out.shape, out.dtype)


# revision 14
# speedup vs baseline: 1.3682x; 1.0224x over previous
"""TRN2 Bass kernel for nn_CompiledBlock_45148696216108 (moe_routing).

Reference computation:
    xp  = x[permute]
    xn  = LayerNorm(xp; gamma, beta, eps=1e-5)
    yp  = xn @ W.T + b
    out = (xp + yp)[argsort(permute)]

The block (LayerNorm + Linear + residual) is purely row-wise, so the
gather by `permute` and the scatter by its inverse cancel exactly:
    out = x + LN(x) @ W.T + b
No token movement (and no cross-core all-to-all) is needed. Tokens are
sharded contiguously across the 8 cores; the tiny weights are folded on
the host and replicated:
    A[h, o] = gamma[h] * W[o, h]          (pre-transposed, gamma folded)
    c[o]    = sum_h beta[h] * W[o, h] + b[o]
    out     = x + ((x - mu) * rsig) @ A + c

Per-core device pipeline (tokens_per_core = 8192, HIDDEN = 2048):
  - DMA x tile (128 tokens, 2048) in natural layout
  - DVE bn_stats/bn_aggr -> mean/var; ACT sqrt + DVE reciprocal -> rsig
  - ACT: xn = (x - mu) * rsig  (per-partition scale/bias)
  - PE transposes xn into (h, t) layout (fp32, via identity)
  - PE matmuls in float32r (full-rate fp32, ~1e-4 rel err):
    psum_j[t, o512] = bias + sum_k xnT[k].T @ A[k, j]
  - DVE per-bank: out_j = x_j + psum_j; DMA out
Stats/normalize for tile i+1 are emitted before tile i's matmul phase so
ACT/DVE prep overlaps the PE matmul stream (keeps PE dense / HAM warm).
"""

import numpy as np
from contextlib import ExitStack

from concourse import bacc, tile, mybir
from concourse.bass_utils import run_bass_kernel_spmd
from concourse.masks import make_identity

N_TOK = 65536
HIDDEN = 2048
N_CORES = 8
P = 128
EPS = 1e-5
F32 = mybir.dt.float32
F32R = mybir.dt.float32r
AF = mybir.ActivationFunctionType
ALU = mybir.AluOpType

NB = 512               # matmul free dim (one PSUM bank of fp32)
KC = HIDDEN // P       # 16 contraction chunks
OC = HIDDEN // NB      # 4 output column chunks


def build(tokens_per_core: int = N_TOK // N_CORES, num_devices: int = N_CORES):
    T = tokens_per_core
    NT = T // P            # token tiles

    nc = bacc.Bacc(
        "TRN2", target_bir_lowering=False, debug=False, num_devices=num_devices
    )
    x_d = nc.dram_tensor("x", [T, HIDDEN], F32, kind="ExternalInput").ap()
    a_d = nc.dram_tensor("A", [KC, P, HIDDEN], F32R, kind="ExternalInput").ap()
    c_d = nc.dram_tensor("c", [P, HIDDEN], F32, kind="ExternalInput").ap()
    out_d = nc.dram_tensor("out", [T, HIDDEN], F32, kind="ExternalOutput").ap()

    with tile.TileContext(nc) as tc, ExitStack() as ctx:
        const = ctx.enter_context(tc.tile_pool(name="const", bufs=1))
        apool = ctx.enter_context(tc.tile_pool(name="apool", bufs=1))
        xpool = ctx.enter_context(tc.tile_pool(name="xpool", bufs=2))
        xnpool = ctx.enter_context(tc.tile_pool(name="xnpool", bufs=2))
        xtpool = ctx.enter_context(tc.tile_pool(name="xtpool", bufs=1))
        outpool = ctx.enter_context(tc.tile_pool(name="outpool", bufs=2))
        stpool = ctx.enter_context(tc.tile_pool(name="stats", bufs=3))
        psy_pool = ctx.enter_context(tc.tile_pool(name="psy", bufs=1, space="PSUM"))
        pst_pool = ctx.enter_context(tc.tile_pool(name="pst", bufs=4, space="PSUM"))

        # Prefetch the first x tiles BEFORE the 16 MB weight DMA so the
        # LN/normalize prologue runs during the weight load.
        xts = {}
        xts[0] = xpool.tile([P, HIDDEN], F32, tag="xt", name="xt_pre0")
        nc.sync.dma_start(xts[0][:], x_d[0:P, :])
        c_sb = const.tile([P, HIDDEN], F32)
        nc.sync.dma_start(c_sb[:], c_d[:])
        if NT > 1:
            xts[1] = xpool.tile([P, HIDDEN], F32, tag="xt", name="xt_pre1")
            nc.sync.dma_start(xts[1][:], x_d[P : 2 * P, :])

        # Resident weights: 16 separately-tracked chunks of (128 h, 2048 o)
        # so matmuls on chunk k only wait for chunk k's DMA.
        a_sb = []
        for k in range(KC):
            ak = apool.tile([P, HIDDEN], F32R, tag=f"a{k}")
            nc.sync.dma_start(ak[:], a_d[k])
            a_sb.append(ak)
        ident_f32 = const.tile([P, P], F32)
        make_identity(nc, ident_f32[:])
        ident = const.tile([P, P], F32R)
        nc.vector.tensor_copy(ident[:], ident_f32[:])
        eps_sb = const.tile([P, 1], F32)
        nc.gpsimd.memset(eps_sb[:], EPS)

        def stats_and_norm(i, xt):
            """LN stats + normalize for tile i -> xn tile (ACT/DVE work)."""
            stats = stpool.tile([P, 4, 6], F32, tag="stats")
            xr = xt[:].rearrange("p (a b) -> p a b", b=512)
            for a in range(4):
                nc.vector.bn_stats(stats[:, a, :], xr[:, a, :])
            mv = stpool.tile([P, 2], F32, tag="mv")
            nc.vector.bn_aggr(mv[:], stats[:])
            sig = stpool.tile([P, 1], F32, tag="sig")
            nc.scalar.activation(sig[:], mv[:, 1:2], AF.Sqrt, bias=eps_sb[:])
            rsig = stpool.tile([P, 1], F32, tag="rsig")
            nc.vector.reciprocal(rsig[:], sig[:])
            nmr = stpool.tile([P, 1], F32, tag="nmr")
            nc.vector.scalar_tensor_tensor(
                nmr[:], mv[:, 0:1], -1.0, rsig[:], ALU.mult, ALU.mult
            )
            xn = xnpool.tile([P, HIDDEN], F32R, tag="xn")
            nc.scalar.activation(xn[:], xt[:], AF.Identity, bias=nmr[:], scale=rsig[:])
            # Pre-bias the residual in place (WAR on the norm read above):
            # combine later does out = psy + (x + c) in one DVE pass.
            nc.gpsimd.tensor_add(xt[:], xt[:], c_sb[:])
            return xn

        xns = {}
        xns[0] = stats_and_norm(0, xts[0])

        for t in range(NT):
            xt = xts.pop(t)
            xn = xns.pop(t)

            # Transpose xn -> xnT (h on partitions), 4 chunks per PSUM bank.
            # One xnT tile per group of 4 chunks so the first matmuls only
            # wait on the first group's PSUM->SBUF cast, not all four.
            xnt_g = []
            for g in range(KC // 4):
                pst = pst_pool.tile([P, 4 * P], F32R, tag="pst")
                for j in range(4):
                    k = 4 * g + j
                    nc.tensor.transpose(
                        pst[:, j * P : (j + 1) * P],
                        xn[:, k * P : (k + 1) * P],
                        ident[:],
                    )
                xg = xtpool.tile([P, 4, P], F32R, tag=f"xnt{g}", name=f"xnt_{t}_{g}")
                nc.vector.tensor_copy(xg[:], pst[:])
                xnt_g.append(xg)

            # Prefetch + prep next tile while this tile's matmuls run
            if t + 2 < NT:
                xts[t + 2] = xpool.tile([P, HIDDEN], F32, tag="xt", name=f"xt_{t + 2}")
                nc.sync.dma_start(
                    xts[t + 2][:], x_d[(t + 2) * P : (t + 3) * P, :]
                )
            if t + 1 < NT:
                xns[t + 1] = stats_and_norm(t + 1, xts[t + 1])  # noqa

            # Matmuls, one PSUM bank (512 outputs) at a time; combine per bank
            ot = outpool.tile([P, HIDDEN], F32, tag="ot")
            for j in range(OC):
                psy = psy_pool.tile([P, NB], F32, tag=f"psy{j}")
                sl = slice(j * NB, (j + 1) * NB)
                for k in range(KC):
                    nc.tensor.matmul(
                        psy[:],
                        xnt_g[k // 4][:, k % 4, :],
                        a_sb[k][:, sl],
                        start=(k == 0),
                        stop=(k == KC - 1),
                    )
                nc.vector.tensor_add(ot[:, sl], psy[:], xt[:, sl])

            nc.sync.dma_start(out_d[t * P : (t + 1) * P, :], ot[:])

    nc.compile()
    return nc


_built = None


def _get_built():
    global _built
    if _built is None:
        _built = build()
    return _built


def _prep_inputs(x, permute, gamma, beta, W, b):
    x = np.asarray(x, dtype=np.float32)
    gamma = np.asarray(gamma, dtype=np.float32)
    beta = np.asarray(beta, dtype=np.float32)
    W = np.asarray(W, dtype=np.float32)
    b = np.asarray(b, dtype=np.float32)
    A = np.ascontiguousarray(W.T) * gamma[:, None]  # (H, O), gamma folded
    A = np.ascontiguousarray(A.reshape(HIDDEN // P, P, HIDDEN))
    c = np.ascontiguousarray(
        np.broadcast_to((W @ beta + b).reshape(1, HIDDEN), (P, HIDDEN))
    ).astype(np.float32)
    T = N_TOK // N_CORES
    in_maps = []
    for i in range(N_CORES):
        in_maps.append({"x": x[i * T : (i + 1) * T], "A": A, "c": c})
    return in_maps


def kernel(x, permute, gamma, beta, W, b):
    nc = _get_built()
    in_maps = _prep_inputs(x, permute, gamma, beta, W, b)
    res = run_bass_kernel_spmd(nc, in_maps, list(range(N_CORES))).results
    return np.concatenate([r["out"] for r in res], axis=0)


if __name__ == "__main__":
    rng = np.random.default_rng(0)
    x = rng.standard_normal((N_TOK, HIDDEN), dtype=np.float32)
    permute = rng.permutation(N_TOK).astype(np.int64)
    gamma = np.ones(HIDDEN, np.float32)
    beta = np.zeros(HIDDEN, np.float32)
    W = (rng.standard_normal((HIDDEN, HIDDEN), dtype=np.float32) / np.sqrt(HIDDEN))
    b = rng.standard_normal(HIDDEN, dtype=np.float32) * 0.01
    out = kernel(x=x, permute=permute, gamma=gamma, beta=beta, W=W, b=b)
    print(out.shape, out.dtype)


# revision 15
# speedup vs baseline: 1.3790x; 1.0079x over previous
"""TRN2 Bass kernel for nn_CompiledBlock_45148696216108 (moe_routing).

Reference computation:
    xp  = x[permute]
    xn  = LayerNorm(xp; gamma, beta, eps=1e-5)
    yp  = xn @ W.T + b
    out = (xp + yp)[argsort(permute)]

The block (LayerNorm + Linear + residual) is purely row-wise, so the
gather by `permute` and the scatter by its inverse cancel exactly:
    out = x + LN(x) @ W.T + b
No token movement (and no cross-core all-to-all) is needed. Tokens are
sharded contiguously across the 8 cores; the tiny weights are folded on
the host and replicated:
    A[h, o] = gamma[h] * W[o, h]          (pre-transposed, gamma folded)
    c[o]    = sum_h beta[h] * W[o, h] + b[o]
    out     = x + ((x - mu) * rsig) @ A + c

Per-core device pipeline (tokens_per_core = 8192, HIDDEN = 2048):
  - DMA x tile (128 tokens, 2048) in natural layout
  - DVE bn_stats/bn_aggr -> mean/var; ACT sqrt + DVE reciprocal -> rsig
  - ACT: xn = (x - mu) * rsig  (per-partition scale/bias)
  - PE transposes xn into (h, t) layout (fp32, via identity)
  - PE matmuls in float32r (full-rate fp32, ~1e-4 rel err):
    psum_j[t, o512] = sum_k xnT[k].T @ A[k, j]
  - GPSIMD pre-biases the residual (x += c broadcast) off-path
  - DVE per-bank: out_j = (x + c)_j + psum_j; DMA out
Stats/normalize for tile i+1 are emitted before tile i's matmul phase so
ACT/DVE prep overlaps the PE matmul stream (keeps PE dense / HAM warm).
"""

import numpy as np
from contextlib import ExitStack

from concourse import bacc, tile, mybir
from concourse.bass_utils import run_bass_kernel_spmd
from concourse.masks import make_identity

N_TOK = 65536
HIDDEN = 2048
N_CORES = 8
P = 128
EPS = 1e-5
F32 = mybir.dt.float32
F32R = mybir.dt.float32r
AF = mybir.ActivationFunctionType
ALU = mybir.AluOpType

NB = 512               # matmul free dim (one PSUM bank of fp32)
KC = HIDDEN // P       # 16 contraction chunks
OC = HIDDEN // NB      # 4 output column chunks


def build(tokens_per_core: int = N_TOK // N_CORES, num_devices: int = N_CORES):
    T = tokens_per_core
    NT = T // P            # token tiles

    nc = bacc.Bacc(
        "TRN2", target_bir_lowering=False, debug=False, num_devices=num_devices
    )
    x_d = nc.dram_tensor("x", [T, HIDDEN], F32, kind="ExternalInput").ap()
    a_d = nc.dram_tensor("A", [KC, P, HIDDEN], F32R, kind="ExternalInput").ap()
    c_d = nc.dram_tensor("c", [P, HIDDEN], F32, kind="ExternalInput").ap()
    out_d = nc.dram_tensor("out", [T, HIDDEN], F32, kind="ExternalOutput").ap()

    with tile.TileContext(nc) as tc, ExitStack() as ctx:
        const = ctx.enter_context(tc.tile_pool(name="const", bufs=1))
        apool = ctx.enter_context(tc.tile_pool(name="apool", bufs=1))
        xpool = ctx.enter_context(tc.tile_pool(name="xpool", bufs=2))
        xnpool = ctx.enter_context(tc.tile_pool(name="xnpool", bufs=2))
        xtpool = ctx.enter_context(tc.tile_pool(name="xtpool", bufs=1))
        outpool = ctx.enter_context(tc.tile_pool(name="outpool", bufs=2))
        stpool = ctx.enter_context(tc.tile_pool(name="stats", bufs=3))
        psy_pool = ctx.enter_context(tc.tile_pool(name="psy", bufs=1, space="PSUM"))
        pst_pool = ctx.enter_context(tc.tile_pool(name="pst", bufs=4, space="PSUM"))

        # Prefetch the first x tiles BEFORE the 16 MB weight DMA so the
        # LN/normalize prologue runs during the weight load.
        xts = {}
        xts[0] = xpool.tile([P, HIDDEN], F32, tag="xt", name="xt_pre0")
        nc.sync.dma_start(xts[0][:], x_d[0:P, :])
        c_sb = const.tile([P, HIDDEN], F32)
        nc.sync.dma_start(c_sb[:], c_d[:])
        if NT > 1:
            xts[1] = xpool.tile([P, HIDDEN], F32, tag="xt", name="xt_pre1")
            nc.sync.dma_start(xts[1][:], x_d[P : 2 * P, :])

        # Resident weights: 16 separately-tracked chunks of (128 h, 2048 o)
        # so matmuls on chunk k only wait for chunk k's DMA.
        a_sb = []
        for k in range(KC):
            ak = apool.tile([P, HIDDEN], F32R, tag=f"a{k}")
            nc.sync.dma_start(ak[:], a_d[k])
            a_sb.append(ak)
        ident_f32 = const.tile([P, P], F32)
        make_identity(nc, ident_f32[:])
        ident = const.tile([P, P], F32R)
        nc.vector.tensor_copy(ident[:], ident_f32[:])
        eps_sb = const.tile([P, 1], F32)
        nc.gpsimd.memset(eps_sb[:], EPS)

        def stats_and_norm(i, xt):
            """LN stats + normalize for tile i -> xn tile (ACT/DVE work)."""
            stats = stpool.tile([P, 4, 6], F32, tag="stats")
            xr = xt[:].rearrange("p (a b) -> p a b", b=512)
            for a in range(4):
                nc.vector.bn_stats(stats[:, a, :], xr[:, a, :])
            mv = stpool.tile([P, 2], F32, tag="mv")
            nc.vector.bn_aggr(mv[:], stats[:])
            sig = stpool.tile([P, 1], F32, tag="sig")
            nc.scalar.activation(sig[:], mv[:, 1:2], AF.Sqrt, bias=eps_sb[:])
            rsig = stpool.tile([P, 1], F32, tag="rsig")
            nc.vector.reciprocal(rsig[:], sig[:])
            nmr = stpool.tile([P, 1], F32, tag="nmr")
            nc.vector.scalar_tensor_tensor(
                nmr[:], mv[:, 0:1], -1.0, rsig[:], ALU.mult, ALU.mult
            )
            xn = xnpool.tile([P, HIDDEN], F32R, tag="xn")
            nc.scalar.activation(xn[:], xt[:], AF.Identity, bias=nmr[:], scale=rsig[:])
            # Pre-bias the residual in place (WAR on the norm read above):
            # combine later does out = psy + (x + c) in one DVE pass.
            nc.gpsimd.tensor_add(xt[:], xt[:], c_sb[:])
            return xn

        xns = {}
        xns[0] = stats_and_norm(0, xts[0])

        for t in range(NT):
            xt = xts.pop(t)
            xn = xns.pop(t)

            # Transpose xn -> xnT (h on partitions), 4 chunks per PSUM bank.
            # One xnT tile per group of 4 chunks so the first matmuls only
            # wait on the first group's PSUM->SBUF cast, not all four.
            xnt_g = []
            for g in range(KC // 4):
                pst = pst_pool.tile([P, 4 * P], F32R, tag="pst")
                for j in range(4):
                    k = 4 * g + j
                    nc.tensor.transpose(
                        pst[:, j * P : (j + 1) * P],
                        xn[:, k * P : (k + 1) * P],
                        ident[:],
                    )
                xg = xtpool.tile([P, 4, P], F32R, tag=f"xnt{g}", name=f"xnt_{t}_{g}")
                nc.vector.tensor_copy(xg[:], pst[:])
                xnt_g.append(xg)

            # Prefetch + prep next tile while this tile's matmuls run
            if t + 2 < NT:
                xts[t + 2] = xpool.tile([P, HIDDEN], F32, tag="xt", name=f"xt_{t + 2}")
                nc.sync.dma_start(
                    xts[t + 2][:], x_d[(t + 2) * P : (t + 3) * P, :]
                )
            if t + 1 < NT:
                xns[t + 1] = stats_and_norm(t + 1, xts[t + 1])  # noqa

            # Matmuls, one PSUM bank (512 outputs) at a time; combine per
            # bank. Tile 0 runs k-outer so each weight chunk is consumed as
            # soon as its DMA lands (overlaps the 16 MB A fill); steady-state
            # tiles run j-outer so each bank completes early for its combine.
            ot = outpool.tile([P, HIDDEN], F32, tag="ot")
            psys = [psy_pool.tile([P, NB], F32, tag=f"psy{j}", name=f"psy_{t}_{j}")
                    for j in range(OC)]
            order = (
                [(j, k) for k in range(KC) for j in range(OC)]
                if t == 0
                else [(j, k) for j in range(OC) for k in range(KC)]
            )
            for j, k in order:
                nc.tensor.matmul(
                    psys[j][:],
                    xnt_g[k // 4][:, k % 4, :],
                    a_sb[k][:, j * NB : (j + 1) * NB],
                    start=(k == 0),
                    stop=(k == KC - 1),
                )
                if k == KC - 1:
                    sl = slice(j * NB, (j + 1) * NB)
                    nc.vector.tensor_add(ot[:, sl], psys[j][:], xt[:, sl])

            nc.sync.dma_start(out_d[t * P : (t + 1) * P, :], ot[:])

    nc.compile()
    return nc


_built = None


def _get_built():
    global _built
    if _built is None:
        _built = build()
    return _built


def _prep_inputs(x, permute, gamma, beta, W, b):
    x = np.asarray(x, dtype=np.float32)
    gamma = np.asarray(gamma, dtype=np.float32)
    beta = np.asarray(beta, dtype=np.float32)
    W = np.asarray(W, dtype=np.float32)
    b = np.asarray(b, dtype=np.float32)
    A = np.ascontiguousarray(W.T) * gamma[:, None]  # (H, O), gamma folded
    A = np.ascontiguousarray(A.reshape(HIDDEN // P, P, HIDDEN))
    c = np.ascontiguousarray(
        np.broadcast_to((W @ beta + b).reshape(1, HIDDEN), (P, HIDDEN))
    ).astype(np.float32)
    T = N_TOK // N_CORES
    in_maps = []
    for i in range(N_CORES):
        in_maps.append({"x": x[i * T : (i + 1) * T], "A": A, "c": c})
    return in_maps


def kernel(x, permute, gamma, beta, W, b):
    nc = _get_built()
    in_maps = _prep_inputs(x, permute, gamma, beta, W, b)
    res = run_bass_kernel_spmd(nc, in_maps, list(range(N_CORES))).results
    return np.concatenate([r["out"] for r in res], axis=0)


if __name__ == "__main__":
    rng = np.random.default_rng(0)
    x = rng.standard_normal((N_TOK, HIDDEN), dtype=np.float32)
    permute = rng.permutation(N_TOK).astype(np.int64)
    gamma = np.ones(HIDDEN, np.float32)
    beta = np.zeros(HIDDEN, np.float32)
    W = (rng.standard_normal((HIDDEN, HIDDEN), dtype=np.float32) / np.sqrt(HIDDEN))
    b = rng.standard_normal(HIDDEN, dtype=np.float32) * 0.01
    out = kernel(x=x, permute=permute, gamma=gamma, beta=beta, W=W, b=b)
    print(out.shape, out.dtype)
